# revision 1
# baseline (speedup 1.0000x reference)
"""Trainium2 Bass kernel for nn_EndPredictor (LN-GRU over B=256,T=512,D=1024,U=1024).

Data-parallel over batch: 32 batch rows per core x 8 cores. Per core:
  Phase 1: S1 = affine(LN(x@W)) as a (T*32, 1024)x(1024, 3072) GEMM in
           float32r, all affine constants (0.2 hard-sigmoid scale, be0, be1,
           +0.5, /g1 fold) baked in; written to DRAM in the scan's folded
           "F2" layout (partition = 32*block + batch, 512-wide free dim).
  Phase 2: T-step recurrent scan. Per step: z|r gate matmul vs U with 4-way
           TensorE column tiling, LayerNorm via bn_stats + cross-partition
           combine/broadcast matmuls, fused affine_then_add LN tail,
           hard-sigmoid clip, candidate matmul, tanh, gated h update, and
           accumulating PE transposes producing h^T for the next step.
  Final:   out = sigmoid(h_T @ W1 + b1) on device.
"""
import sys
for _p in ("/opt/trn_rl_repo", "/root/.axon_site/_ro/trn_rl_repo"):
    if _p not in sys.path:
        sys.path.insert(0, _p)

import numpy as np
import concourse.bass as bass
import concourse.bacc as bacc
import concourse.tile as tile
from concourse import mybir
from concourse.bass_utils import run_bass_kernel_spmd
from contextlib import ExitStack

F32 = mybir.dt.float32
F32R = mybir.dt.float32r
BF16 = mybir.dt.float16
AF = mybir.ActivationFunctionType
OP = mybir.AluOpType

B, D, UNITS = 256, 1024, 1024
U3 = 3 * UNITS
Z2 = 2 * UNITS
NCORES = 8
BC = B // NCORES
EPS = 1e-5

# Tunables (test scripts may override before calling kernel())
T_STEPS = 512
TRACE = False
DBG_SCAN_STEPS = None   # limit scan steps (debug)
DBG_SKIP_P1 = False     # skip phase-1 body (debug)


def build_program(T, b1val, apply_mask=False, has_b=False):
    nc = bacc.Bacc("TRN2", target_bir_lowering=False, debug=False,
                   num_devices=NCORES)
    R = BC * T

    xt = nc.dram_tensor("xt", [D, R], F32R, kind="ExternalInput")
    w = nc.dram_tensor("w", [D, U3], F32R, kind="ExternalInput")
    u = nc.dram_tensor("u", [D, U3], BF16, kind="ExternalInput")
    afold = nc.dram_tensor("afold", [U3], F32, kind="ExternalInput")
    cfold = nc.dram_tensor("cfold", [U3], F32, kind="ExternalInput")
    gz = nc.dram_tensor("gz", [128, 512], F32, kind="ExternalInput")
    gc = nc.dram_tensor("gc", [64, 512], F32, kind="ExternalInput")
    a32 = nc.dram_tensor("a32", [128, 32], F32, kind="ExternalInput")
    bc32 = nc.dram_tensor("bc32", [32, 128], F32, kind="ExternalInput")
    id128 = nc.dram_tensor("id128", [128, 32], F32, kind="ExternalInput")
    w1f = nc.dram_tensor("w1f", [64, 512], F32, kind="ExternalInput")
    if has_b:
        bvec = nc.dram_tensor("bvec", [U3], F32, kind="ExternalInput")
    if apply_mask:
        mz = nc.dram_tensor("mz", [T, 64, 2], F32, kind="ExternalInput")
    s1z = nc.dram_tensor("s1z", [T, 128, 512], F32, kind="Internal")
    s1c = nc.dram_tensor("s1c", [T, 64, 512], F32, kind="Internal")
    out = nc.dram_tensor("out", [BC, 1], F32, kind="ExternalOutput")
    hdbg = nc.dram_tensor("hdbg", [64, 512], F32, kind="ExternalOutput")

    xt_r = xt.ap().rearrange("(k p) r -> p k r", k=8)
    w_r = w.ap().rearrange("(k p) c -> p k c", k=8)
    u_r = u.ap().rearrange("(k p) c -> p k c", k=8)

    def bcast_ap(h, n, cols):
        return bass.AP(tensor=h.ap().tensor, offset=0, ap=[[0, n], [1, cols]])

    with tile.TileContext(nc) as tc:
        # =================== PHASE 1 ===================
        ntiles = R // 128
        with ExitStack() as p1:
            wpool = p1.enter_context(tc.tile_pool(name="wpool", bufs=1))
            xpool = p1.enter_context(tc.tile_pool(name="xpool", bufs=3))
            opool = p1.enter_context(tc.tile_pool(name="opool", bufs=3))
            stpool = p1.enter_context(tc.tile_pool(name="stp1", bufs=3))
            pp = p1.enter_context(tc.tile_pool(name="pp1", bufs=1, space="PSUM"))

            wsb = wpool.tile([128, 8, U3], F32R)
            nc.sync.dma_start(out=wsb, in_=w_r)
            epsb1 = wpool.tile([128, 1], F32)
            nc.vector.memset(epsb1, EPS)
            afb = wpool.tile([128, U3], F32)
            nc.sync.dma_start(out=afb, in_=bcast_ap(afold, 128, U3))
            cfb = wpool.tile([128, U3], F32)
            nc.sync.dma_start(out=cfb, in_=bcast_ap(cfold, 128, U3))
            if has_b:
                bvb = wpool.tile([128, U3], F32)
                nc.sync.dma_start(out=bvb, in_=bcast_ap(bvec, 128, U3))

            for rt in range(0 if DBG_SKIP_P1 else ntiles):
                xts = xpool.tile([128, 8, 128], F32R, tag="xts")
                nc.sync.dma_start(out=xts, in_=xt_r[:, :, rt * 128:(rt + 1) * 128])
                psums = []
                for half in range(3):
                    ps = pp.tile([128, 1024], F32, tag=f"p1ps{half}")
                    psums.append(ps)
                    for sub in range(2):
                        n = 2 * half + sub
                        for k in range(8):
                            nc.tensor.matmul(
                                ps[:, sub * 512:(sub + 1) * 512],
                                xts[:, k, :],
                                wsb[:, k, n * 512:(n + 1) * 512],
                                start=(k == 0), stop=(k == 7))
                if has_b:
                    for half in range(3):
                        nc.vector.tensor_tensor(
                            out=psums[half], in0=psums[half],
                            in1=bvb[:, half * 1024:(half + 1) * 1024], op=OP.add)
                stats = stpool.tile([128, 6, 6], F32, tag="stats")
                for n in range(6):
                    nc.vector.bn_stats(
                        out=stats[:, n, :],
                        in_=psums[n // 2][:, (n % 2) * 512:(n % 2 + 1) * 512])
                mv = stpool.tile([128, 2], F32, tag="mv")
                nc.vector.bn_aggr(out=mv, in_=stats)
                sd = stpool.tile([128, 1], F32, tag="sd")
                nc.scalar.activation(out=sd, in_=mv[:, 1:2], func=AF.Sqrt,
                                     bias=epsb1, scale=1.0)
                nc.vector.tensor_scalar_add(out=sd, in0=sd, scalar1=EPS)
                rv = stpool.tile([128, 2], F32, tag="rv")
                nc.vector.reciprocal(out=rv[:, 1:2], in_=sd)
                nc.vector.tensor_scalar(out=rv[:, 0:1], in0=mv[:, 0:1],
                                        scalar1=rv[:, 1:2], scalar2=-1.0,
                                        op0=OP.mult, op1=OP.mult)
                t0 = 4 * rt
                for n in range(6):
                    sb = opool.tile([128, 512], F32, tag=f"o{n % 3}")
                    nc.vector.affine_then_add(
                        out=sb,
                        in0=psums[n // 2][:, (n % 2) * 512:(n % 2 + 1) * 512],
                        in1=cfb[:, n * 512:(n + 1) * 512],
                        scale=rv[:, 1:2], bias=rv[:, 0:1])
                    nc.vector.tensor_tensor(out=sb, in0=sb,
                                            in1=afb[:, n * 512:(n + 1) * 512],
                                            op=OP.mult)
                    for dt in range(4):
                        if n < 4:
                            dst = s1z.ap()[t0 + dt, 32 * n:32 * n + 32, :]
                        else:
                            dst = s1c.ap()[t0 + dt, 32 * (n - 4):32 * (n - 4) + 32, :]
                        nc.sync.dma_start(out=dst, in_=sb[32 * dt:32 * dt + 32, :])

        # =================== PHASE 2 ===================
        with ExitStack() as p2:
            cons = p2.enter_context(tc.tile_pool(name="cons", bufs=1))
            sp = p2.enter_context(tc.tile_pool(name="scan_sb", bufs=2))
            s1p = p2.enter_context(tc.tile_pool(name="s1p", bufs=3))
            hp = p2.enter_context(tc.tile_pool(name="hp", bufs=2))
            pz = p2.enter_context(tc.tile_pool(name="pz", bufs=1, space="PSUM"))
            pc = p2.enter_context(tc.tile_pool(name="pc", bufs=1, space="PSUM"))
            pt = p2.enter_context(tc.tile_pool(name="pt", bufs=1, space="PSUM"))
            pst = p2.enter_context(tc.tile_pool(name="pst", bufs=2, space="PSUM"))

            usb = cons.tile([128, 8, U3], BF16)
            nc.sync.dma_start(out=usb, in_=u_r)
            gzsb = cons.tile([128, 512], F32)
            nc.sync.dma_start(out=gzsb, in_=gz.ap())
            gcsb = cons.tile([64, 512], F32)
            nc.sync.dma_start(out=gcsb, in_=gc.ap())
            a32sb = cons.tile([128, 32], F32)
            nc.sync.dma_start(out=a32sb, in_=a32.ap())
            bc32sb = cons.tile([32, 128], F32)
            nc.sync.dma_start(out=bc32sb, in_=bc32.ap())
            idsb = cons.tile([128, 32], F32)
            nc.sync.dma_start(out=idsb, in_=id128.ap())
            w1sb = cons.tile([64, 512], F32)
            nc.sync.dma_start(out=w1sb, in_=w1f.ap())
            epsb = cons.tile([32, 1], F32)
            nc.vector.memset(epsb, EPS)
            b1b = cons.tile([32, 1], F32)
            nc.vector.memset(b1b, float(b1val))

            h = hp.tile([64, 512], F32, tag="h")
            hT = hp.tile([128, 256], BF16, tag="hT")
            nc.vector.memset(h, 0.0)
            z0 = sp.tile([128, 256], F32, tag="z0")
            nc.vector.memset(z0, 0.0)
            nc.vector.tensor_copy(out=hT, in_=z0)

            def trans8(dst_ps, src0, src1):
                """8 PE transposes of a (64,512)-F2 tensor into (128, 256).
                src0 = the tile (j=0 half read at base 0), src1 = staged
                base-0 copy of partitions 32:64."""
                for k in range(8):
                    j, hf = divmod(k, 4)
                    srcap = (src0[0:32, 128 * hf:128 * hf + 128] if j == 0
                             else src1[:, 128 * hf:128 * hf + 128])
                    nc.tensor.matmul(
                        dst_ps[:, 32 * k:32 * k + 32],
                        srcap, idsb[0:32, :],
                        is_transpose=True, start=True, stop=True)

            def ln_stats(ps, nparts, ngroups, a_mat, bc_mat, tag):
                """LayerNorm stats for folded psum (nparts, 512) with ngroups
                partition groups per batch row. Returns SBUF (nparts, 2) tile
                [(-mean*rinv), rinv] broadcast to all partition groups."""
                st = sp.tile([nparts, 6], F32, tag=f"{tag}st")
                nc.vector.bn_stats(out=st, in_=ps)
                mv = sp.tile([nparts, 2], F32, tag=f"{tag}mv")
                nc.vector.bn_aggr(out=mv, in_=st)
                msq = sp.tile([nparts, 1], F32, tag=f"{tag}msq")
                nc.vector.tensor_tensor(out=msq, in0=mv[:, 0:1], in1=mv[:, 0:1],
                                        op=OP.mult)
                cp = pst.tile([32, 3], F32, tag="cstat")
                nc.tensor.matmul(cp[:, 0:2], a_mat, mv, start=True, stop=True)
                nc.tensor.matmul(cp[:, 2:3], a_mat, msq, start=True, stop=True)
                cpsb = sp.tile([32, 3], F32, tag=f"{tag}cpsb")
                nc.vector.tensor_copy(out=cpsb, in_=cp)
                inv_n = 1.0 / ngroups
                ex2 = sp.tile([32, 4], F32, tag=f"{tag}ex2")
                nc.vector.tensor_scalar(out=ex2[:, 0:1], in0=cpsb[:, 1:2],
                                        scalar1=cpsb[:, 2:3], scalar2=inv_n,
                                        op0=OP.add, op1=OP.mult)
                nc.vector.tensor_scalar(out=ex2[:, 1:2], in0=cpsb[:, 0:1],
                                        scalar1=cpsb[:, 0:1], scalar2=inv_n * inv_n,
                                        op0=OP.mult, op1=OP.mult)
                nc.vector.tensor_tensor(out=ex2[:, 2:3], in0=ex2[:, 0:1],
                                        in1=ex2[:, 1:2], op=OP.subtract)
                sd = sp.tile([32, 1], F32, tag=f"{tag}sd")
                nc.scalar.activation(out=sd, in_=ex2[:, 2:3], func=AF.Sqrt,
                                     bias=epsb, scale=1.0)
                nc.vector.tensor_scalar_add(out=sd, in0=sd, scalar1=EPS)
                mr = sp.tile([32, 2], F32, tag=f"{tag}mr")
                nc.vector.reciprocal(out=mr[:, 1:2], in_=sd)
                nc.vector.tensor_scalar(out=mr[:, 0:1], in0=cpsb[:, 0:1],
                                        scalar1=mr[:, 1:2], scalar2=-inv_n,
                                        op0=OP.mult, op1=OP.mult)
                bps = pst.tile([128, 2], F32, tag="bcast")
                nc.tensor.matmul(bps[0:nparts, :], bc_mat, mr, start=True, stop=True)
                mrS = sp.tile([nparts, 2], F32, tag=f"{tag}mrS")
                nc.vector.tensor_copy(out=mrS, in_=bps[0:nparts, :])
                return mrS

            nsteps = T if DBG_SCAN_STEPS is None else min(T, DBG_SCAN_STEPS)
            for t in range(nsteps):
                s1zt = s1p.tile([128, 512], F32, tag="s1z")
                nc.sync.dma_start(out=s1zt, in_=s1z.ap()[t])
                s1ct = s1p.tile([64, 512], F32, tag="s1c")
                nc.sync.dma_start(out=s1ct, in_=s1c.ap()[t])

                zrps = pz.tile([128, 512], F32, tag="zrps")
                for k in range(8):
                    for jj in range(4):
                        nc.tensor.matmul(zrps[32 * jj:32 * jj + 32, :],
                                         hT[:, 32 * k:32 * k + 32],
                                         usb[:, k, 512 * jj:512 * jj + 512],
                                         start=(k == 0), stop=(k == 7),
                                         tile_position=(0, 32 * jj),
                                         skip_group_check=True)
                mrS = ln_stats(zrps, 128, 4, a32sb, bc32sb, "z")
                s = sp.tile([128, 512], F32, tag="s")
                nc.vector.affine_then_add(out=s, in0=zrps, in1=s1zt,
                                          scale=mrS[:, 1:2], bias=mrS[:, 0:1])
                nc.vector.tensor_tensor(out=s, in0=s, in1=gzsb, op=OP.mult)
                z = sp.tile([64, 512], F32, tag="z")
                r = sp.tile([64, 512], F32, tag="r")
                nc.vector.tensor_scalar(out=z, in0=s[0:64, :], scalar1=0.0,
                                        scalar2=1.0, op0=OP.max, op1=OP.min)
                nc.vector.tensor_scalar(out=r, in0=s[64:128, :], scalar1=0.0,
                                        scalar2=1.0, op0=OP.max, op1=OP.min)
                if apply_mask:
                    mzt = s1p.tile([64, 2], F32, tag="mzt")
                    nc.sync.dma_start(out=mzt, in_=mz.ap()[t])
                    nc.vector.tensor_scalar(out=z, in0=z,
                                            scalar1=mzt[:, 0:1],
                                            scalar2=mzt[:, 1:2],
                                            op0=OP.mult, op1=OP.add)
                rh = sp.tile([64, 512], F32, tag="rh")
                nc.vector.tensor_tensor(out=rh, in0=r, in1=h, op=OP.mult)
                rh1 = sp.tile([32, 512], F32, tag="rh1")
                nc.vector.tensor_copy(out=rh1, in_=rh[32:64, :])
                rhtps = pt.tile([128, 256], F32, tag="tps")
                trans8(rhtps, rh, rh1)
                rhT = sp.tile([128, 256], BF16, tag="rhT")
                nc.vector.tensor_copy(out=rhT, in_=rhtps)

                cps = pc.tile([64, 512], F32, tag="cps")
                for k in range(8):
                    for j in range(2):
                        nc.tensor.matmul(cps[32 * j:32 * j + 32, :],
                                         rhT[:, 32 * k:32 * k + 32],
                                         usb[:, k, 2048 + 512 * j:2560 + 512 * j],
                                         start=(k == 0), stop=(k == 7),
                                         tile_position=(0, 32 * j),
                                         skip_group_check=True)
                mrSc = ln_stats(cps, 64, 2, a32sb[0:64, :], bc32sb[:, 0:64], "c")
                cs = sp.tile([64, 512], F32, tag="cs")
                nc.vector.affine_then_add(out=cs, in0=cps, in1=s1ct,
                                          scale=mrSc[:, 1:2], bias=mrSc[:, 0:1])
                nc.vector.tensor_tensor(out=cs, in0=cs, in1=gcsb, op=OP.mult)
                th = sp.tile([64, 512], F32, tag="th")
                nc.scalar.activation(out=th, in_=cs, func=AF.Tanh)
                dd = sp.tile([64, 512], F32, tag="dd")
                nc.vector.tensor_tensor(out=dd, in0=h, in1=th, op=OP.subtract)
                nc.vector.tensor_tensor(out=dd, in0=z, in1=dd, op=OP.mult)
                hn = hp.tile([64, 512], F32, tag="h")
                nc.vector.tensor_tensor(out=hn, in0=dd, in1=th, op=OP.add)
                hn1 = sp.tile([32, 512], F32, tag="hn1")
                nc.vector.tensor_copy(out=hn1, in_=hn[32:64, :])
                htps = pt.tile([128, 256], F32, tag="tps")
                trans8(htps, hn, hn1)
                hTn = hp.tile([128, 256], BF16, tag="hT")
                nc.vector.tensor_copy(out=hTn, in_=htps)
                h, hT = hn, hTn

            pscr = sp.tile([64, 512], F32, tag="pscr")
            nc.vector.tensor_tensor(out=pscr, in0=h, in1=w1sb, op=OP.mult)
            pacc = sp.tile([64, 1], F32, tag="pacc")
            nc.vector.tensor_reduce(out=pacc, in_=pscr,
                                    axis=mybir.AxisListType.X, op=OP.add)
            pc1 = sp.tile([32, 1], F32, tag="pc1")
            nc.vector.tensor_copy(out=pc1, in_=pacc[32:64, :])
            ptot = sp.tile([32, 1], F32, tag="ptot")
            nc.vector.tensor_tensor(out=ptot, in0=pacc[0:32, :], in1=pc1,
                                    op=OP.add)
            osb = sp.tile([32, 1], F32, tag="osb")
            nc.scalar.activation(out=osb, in_=ptot, func=AF.Sigmoid,
                                 bias=b1b, scale=1.0)
            nc.sync.dma_start(out=out.ap(), in_=osb)
            nc.sync.dma_start(out=hdbg.ap(), in_=h)

    nc.compile()
    return nc


def _host_prep(x, mask, W, U, b, gammas, betas, W1, b1, T, apply_mask, has_b):
    g0 = np.asarray(gammas[0], np.float64)
    g1 = np.asarray(gammas[1], np.float64)
    be0 = np.asarray(betas[0], np.float64)
    be1 = np.asarray(betas[1], np.float64)
    afold = (g0 / g1).astype(np.float32)
    cc = be0 + be1
    cc = cc.copy()
    cc[:Z2] += 2.5
    cfold = (cc / g0).astype(np.float32)
    gzv = (0.2 * g1[:Z2]).astype(np.float32)
    gcv = g1[Z2:].astype(np.float32)
    gz = np.ascontiguousarray(
        np.broadcast_to(gzv.reshape(4, 1, 512), (4, 32, 512)).reshape(128, 512))
    gc = np.ascontiguousarray(
        np.broadcast_to(gcv.reshape(2, 1, 512), (2, 32, 512)).reshape(64, 512))
    a32 = np.zeros((128, 32), np.float32)
    a32[np.arange(128), np.arange(128) % 32] = 1.0
    bc32 = np.ascontiguousarray(a32.T)
    id128 = np.ascontiguousarray(np.tile(np.eye(32, dtype=np.float32), (4, 1)))
    import ml_dtypes
    shared = dict(w=np.ascontiguousarray(W, np.float32),
                  u=np.ascontiguousarray(np.asarray(U, np.float32).astype(np.float16)),
                  afold=afold, cfold=cfold, gz=gz, gc=gc,
                  a32=a32, bc32=bc32, id128=id128,
                  w1f=np.ascontiguousarray(np.broadcast_to(
                      np.asarray(W1, np.float32).reshape(2, 512)[:, None, :],
                      (2, 32, 512)).reshape(64, 512)))
    if has_b:
        shared["bvec"] = np.ascontiguousarray(b, np.float32)
    in_maps = []
    for c in range(NCORES):
        xc = x[c * BC:(c + 1) * BC, :T, :]
        xtc = np.ascontiguousarray(np.transpose(xc, (2, 1, 0)).reshape(D, T * BC))
        m = dict(shared)
        m["xt"] = xtc
        if apply_mask:
            mc = np.asarray(mask[c * BC:(c + 1) * BC, :T], np.float32)  # (32, T)
            mzt = np.empty((T, 64, 2), np.float32)
            mzt[:, 0:32, 0] = mc.T
            mzt[:, 32:64, 0] = mc.T
            mzt[:, :, 1] = 1.0 - mzt[:, :, 0]
            m["mz"] = mzt
        in_maps.append(m)
    return in_maps


def kernel(x, mask, W, U, b, gammas, betas, W1, b1):
    import time as _time
    x = np.asarray(x)
    T = x.shape[1]
    has_b = bool(np.any(np.asarray(b)))
    apply_mask = not bool(np.asarray(mask).all())
    b1val = float(np.asarray(b1).reshape(-1)[0])
    _t0 = _time.time()
    nc = build_program(T, b1val, apply_mask=apply_mask, has_b=has_b)
    _t1 = _time.time()
    in_maps = _host_prep(x, mask, W, U, b, gammas, betas, W1, b1, T,
                         apply_mask, has_b)
    _t2 = _time.time()
    res = run_bass_kernel_spmd(nc, in_maps, core_ids=list(range(NCORES)),
                               trace=TRACE)
    _t3 = _time.time()
    print(f"[kernel] build {_t1-_t0:.1f}s prep {_t2-_t1:.1f}s run {_t3-_t2:.1f}s")
    kernel.last_result = res
    outs = [res.results[c]["out"].reshape(BC, 1) for c in range(NCORES)]
    return np.concatenate(outs, axis=0).astype(np.float32)

